# revision 1
# baseline (speedup 1.0000x reference)
"""BertCorrector kernel for 8 TRN2 NeuronCores.

Computes: segment-mean merge of subword encodings (sorted per-row segment
ids) followed by a dense vocab projection:
    merged[b,w,:] = mean_{s: ids[b,s]==w} enc[b,s,:]   (0 if empty)
    logits = merged @ W + b

Strategy: data-parallel over batch (4 samples/core).  The segment-mean is
computed on the TensorEngine as enc^T @ S where S is a per-sample one-hot
matrix pre-scaled by 1/count (built host-side from segment_ids).  That
directly yields merged TRANSPOSED ([H, W] chunks), which is exactly the
stationary-operand layout the vocab-projection matmul needs.  All matmul
inputs are bf16 (fp32 PSUM accumulation); the output is written f32.
"""

import numpy as np
import ml_dtypes

B, S, H = 32, 512, 768
V = 8192
WMAX = 256
NCORES = 8
PB = B // NCORES  # samples per core
P = 128

KC = S // P   # 4 token chunks (contraction of stage A)
KO = H // P   # 6 hidden chunks
WT = WMAX // P  # 2 word tiles
NV = 512      # vocab tile
NT = V // NV  # 16 vocab tiles

_compiled = None


def _build_program():
    import concourse.bass as bass
    import concourse.mybir as mybir
    from concourse import bacc
    from concourse.tile import TileContext

    bf16 = mybir.dt.bfloat16
    f32 = mybir.dt.float32

    nc = bacc.Bacc()
    enc_d = nc.dram_tensor("enc", [PB, S, H], bf16, kind="ExternalInput")
    aux_d = nc.dram_tensor("aux", [P, PB, 2, KC], f32, kind="ExternalInput")
    w_d = nc.dram_tensor("wmat", [H, V], bf16, kind="ExternalInput")
    out_d = nc.dram_tensor("out", [PB, WMAX, V], f32, kind="ExternalOutput")

    enc_r = enc_d.rearrange("b (kc p) h -> b p kc h", p=P)
    w_r = w_d.rearrange("(ko p) v -> p ko v", p=P)

    with TileContext(nc) as tc:
        with (
            tc.tile_pool(name="persist", bufs=1) as persist,
            tc.tile_pool(name="encp", bufs=2) as encp,
            tc.tile_pool(name="onehp", bufs=4) as onehp,
            tc.tile_pool(name="wp", bufs=3) as wp,
            tc.tile_pool(name="outp", bufs=8) as outp,
            tc.tile_pool(name="ps1", bufs=2, space="PSUM") as ps1,
            tc.tile_pool(name="ps2", bufs=6, space="PSUM") as ps2,
        ):
            # mergedT[h_in_chunk, ko, s, w] resident in SBUF (bf16)
            mergedT = persist.tile([P, KO, PB, WMAX], bf16)

            # Prefetch the first W chunks so stage B never stalls on them.
            w_tiles = {}

            def load_w(n):
                if n < NT:
                    t = wp.tile([P, KO, NV], bf16, tag="w")
                    nc.sync.dma_start(out=t[:], in_=w_r[:, :, n * NV:(n + 1) * NV])
                    w_tiles[n] = t

            # iota row (0..WMAX-1, identical on every partition), on-device
            iota_sb = persist.tile([P, WMAX], f32)
            nc.gpsimd.iota(
                iota_sb[:], pattern=[[1, WMAX]], base=0,
                channel_multiplier=0, allow_small_or_imprecise_dtypes=True,
            )
            # all samples' (segment id, 1/count) pairs in one contiguous DMA
            aux_sb = persist.tile([P, PB, 2, KC], f32)
            nc.sync.dma_start(out=aux_sb[:], in_=aux_d[:])

            # Warm the PE clock gate while the first input DMAs fly.
            warm_sb = persist.tile([P, P], bf16)
            nc.gpsimd.memset(warm_sb[:], 0.0)
            warm_ps = ps1.tile([P, 64], f32, tag="ps1")
            for _ in range(68):
                nc.tensor.matmul(
                    warm_ps[:], lhsT=warm_sb[:], rhs=warm_sb[:, :64],
                    start=True, stop=True,
                )

            # ---- Stage A: mergedT = enc^T @ scaled_onehot, per sample ----
            # Scaled one-hots are built on-chip for ALL samples up front
            # (DVE runs in emission order, so these never queue behind the
            # mergedT casts): oneh[tok,w] = (iota[w] == ids[tok]) / count
            oneh_tiles = []
            for s in range(PB):
                t = onehp.tile([P, KC, WMAX], bf16, tag="oneh", name=f"oneh{s}")
                for kc in range(KC):
                    nc.vector.tensor_scalar(
                        out=t[:, kc],
                        in0=iota_sb[:],
                        scalar1=aux_sb[:, s, 0, kc:kc + 1],
                        scalar2=aux_sb[:, s, 1, kc:kc + 1],
                        op0=mybir.AluOpType.is_equal,
                        op1=mybir.AluOpType.mult,
                    )
                oneh_tiles.append(t)

            for s in range(PB):
                enc_sb = encp.tile([P, KC, H], bf16, tag="enc")
                if s == 0:
                    # split across DMA queues so the first chunk lands sooner
                    for kc in range(KC):
                        nc.sync.dma_start(out=enc_sb[:, kc], in_=enc_r[s, :, kc])
                else:
                    nc.sync.dma_start(out=enc_sb[:], in_=enc_r[s])
                oneh_sb = oneh_tiles[s]
                if s == 0:
                    # kc-outer for the first sample only: start matmuls as
                    # soon as enc chunk 0 lands instead of waiting for the
                    # whole sample (6 concurrent psum groups from ps2).
                    pts = [
                        ps2.tile([P, NV], f32, tag="ps2", name=f"pa{i}")
                        for i in range(KO)
                    ]
                    for kc in range(KC):
                        for ko in range(KO):
                            nc.tensor.matmul(
                                pts[ko][:, :WMAX],
                                lhsT=enc_sb[:, kc, ko * P:(ko + 1) * P],
                                rhs=oneh_sb[:, kc, :],
                                start=(kc == 0),
                                stop=(kc == KC - 1),
                            )
                    load_w(0)
                    for ko in range(KO):
                        nc.vector.tensor_copy(
                            out=mergedT[:, ko, s, :], in_=pts[ko][:, :WMAX]
                        )
                    continue
                if s == 1:
                    load_w(1)
                for ko in range(KO):
                    pt = ps1.tile([P, WMAX], f32, tag="ps1")
                    for kc in range(KC):
                        nc.tensor.matmul(
                            pt[:],
                            lhsT=enc_sb[:, kc, ko * P:(ko + 1) * P],
                            rhs=oneh_sb[:, kc, :],
                            start=(kc == 0),
                            stop=(kc == KC - 1),
                        )
                    nc.vector.tensor_copy(out=mergedT[:, ko, s, :], in_=pt[:])

            # ---- Stage B: out[s, w, v] = mergedT^T @ W, tiled over vocab ----
            # The last vocab chunk is computed in two half-width passes so
            # the final (unoverlappable) psum-copy + store drain is halved.
            for n in range(NT):
                load_w(n + 2)
                w_sb = w_tiles.pop(n)
                segs = [(0, NV)] if n < NT - 1 else [(0, NV // 2), (NV // 2, NV // 2)]
                for c0, cw in segs:
                    for s in range(PB):
                        for wt in range(WT):
                            pt = ps2.tile([P, NV], f32, tag="ps2")
                            for ko in range(KO):
                                nc.tensor.matmul(
                                    pt[:, :cw],
                                    lhsT=mergedT[:, ko, s, wt * P:(wt + 1) * P],
                                    rhs=w_sb[:, ko, c0:c0 + cw],
                                    start=(ko == 0),
                                    stop=(ko == KO - 1),
                                )
                            ot = outp.tile([P, NV], f32, tag="out")
                            nc.vector.tensor_copy(out=ot[:, :cw], in_=pt[:, :cw])
                            nc.sync.dma_start(
                                out=out_d[s, wt * P:(wt + 1) * P,
                                          n * NV + c0:n * NV + c0 + cw],
                                in_=ot[:, :cw],
                            )

    nc.finalize()
    return nc


def _get_program():
    global _compiled
    if _compiled is None:
        _compiled = _build_program()
    return _compiled


def _prep_inputs(bert_encodings, segment_ids, W):
    enc_bf = np.asarray(bert_encodings, dtype=np.float32).astype(ml_dtypes.bfloat16)
    w_bf = np.asarray(W, dtype=np.float32).astype(ml_dtypes.bfloat16)

    ids = np.asarray(segment_ids).astype(np.int64)
    flat = (ids + np.arange(B, dtype=np.int64)[:, None] * WMAX).ravel()
    counts = np.bincount(flat, minlength=B * WMAX).reshape(B, WMAX)
    inv = (1.0 / np.maximum(counts, 1)).astype(np.float32)

    # per-token (segment id, 1/count) pairs, pre-transposed to the SBUF
    # layout [p, sample, {id,inv}, kc] so each core gets one contiguous DMA
    idsval = np.empty((B, 2, S), dtype=np.float32)
    idsval[:, 0, :] = ids.astype(np.float32)
    idsval[:, 1, :] = np.take_along_axis(inv, ids, axis=1)
    aux = np.ascontiguousarray(
        idsval.reshape(NCORES, PB, 2, KC, P).transpose(0, 4, 1, 2, 3)
    )
    return enc_bf, w_bf, aux


def kernel(bert_encodings, segment_ids, W, b, num_words, _trace=False):
    from concourse.bass_utils import run_bass_kernel_spmd

    assert int(num_words) == WMAX
    enc_bf, w_bf, aux = _prep_inputs(bert_encodings, segment_ids, W)

    nc = _get_program()
    core_ids = list(range(NCORES))
    in_maps = [
        {
            "enc": enc_bf[c * PB:(c + 1) * PB],
            "aux": aux[c],
            "wmat": w_bf,
        }
        for c in core_ids
    ]
    res = run_bass_kernel_spmd(nc, in_maps, core_ids, trace=_trace)
    out = np.concatenate([res.results[c]["out"] for c in core_ids], axis=0)
    out = np.ascontiguousarray(out.reshape(B, WMAX, V))

    bias = np.asarray(b, dtype=np.float32)
    if np.any(bias):
        out = out + bias

    if _trace:
        kernel._last_exec_time_ns = res.exec_time_ns
        kernel._last_result = res
    return out



# revision 2
# speedup vs baseline: 1.2856x; 1.2856x over previous
"""BertCorrector kernel for 8 TRN2 NeuronCores.

Computes: segment-mean merge of subword encodings (sorted per-row segment
ids) followed by a dense vocab projection:
    merged[b,w,:] = mean_{s: ids[b,s]==w} enc[b,s,:]   (0 if empty)
    logits = merged @ W + b

Strategy (v2):
  * Globally pack the non-empty (sample, word) pairs (~86.5% of B*WMAX)
    into one contiguous axis and split it evenly across the 8 cores at
    word granularity.  Each core gets ~1/8 of the packed words plus the
    contiguous token range feeding them.  With the observed fill rate
    this is 7 word-tiles of 128 per core instead of 8 -> 12.5% fewer
    stage-B matmul columns and output bytes.
  * Stage A (segment sum) runs as enc^T @ onehot on the TensorEngine.
    Because tokens are sorted by packed word id, each 128-token chunk
    only touches a narrow window of packed-word columns; the matmul
    streams just that window (computed from the actual ids at build
    time, unioned over cores) instead of all 256 columns.
  * Stage B streams W tiles against the stationary packed mergedT.
    PSUM results are cast to bf16 during the PSUM->SBUF copy
    (alternating Vector/Scalar engines) and written to DRAM as bf16
    with 2 KiB per-partition lines; the host upconverts and scatters
    rows back to the dense [B, WMAX, V] f32 output.
"""

import numpy as np
import ml_dtypes

B, S, H = 32, 512, 768
V = 8192
WMAX = 256
NCORES = 8
P = 128
KO = H // P          # 6 hidden chunks
NVP = 1024           # vocab pair width (2KiB bf16 DMA lines)
NPAIR = V // NVP     # 8 vocab pairs
NWARM = 24


def _plan(segment_ids):
    """Pack non-empty words globally, split across cores, compute windows.

    Returns a dict with everything the program builder + host prep need.
    """
    ids = np.asarray(segment_ids, np.int64)
    tok_pid = np.empty((B, S), np.int64)    # global packed word id per token
    packed_rows = []                        # global row index b*WMAX+w per packed word
    counts = []
    base = 0
    for b in range(B):
        u, inv_idx, cnt = np.unique(ids[b], return_inverse=True, return_counts=True)
        tok_pid[b] = base + inv_idx
        packed_rows.append(b * WMAX + u)
        counts.append(cnt)
        base += len(u)
    T = base
    packed_rows = np.concatenate(packed_rows)
    counts = np.concatenate(counts).astype(np.float64)
    flat_pid = tok_pid.ravel()              # nondecreasing

    wbound = np.array([round(c * T / NCORES) for c in range(NCORES + 1)])
    tbound = np.searchsorted(flat_pid, wbound)
    assert tbound[0] == 0 and tbound[-1] == B * S

    nwords = wbound[1:] - wbound[:-1]
    ntoks = tbound[1:] - tbound[:-1]
    WP = int(-(-nwords.max() // P) * P)     # padded packed words per core
    KC = int(-(-ntoks.max() // P))          # token chunks per core
    PTW = WP // 2                           # psum tile width (<=512)
    assert PTW <= 512

    # per-chunk packed-word windows, unioned over cores
    wins = []
    for kc in range(KC):
        lo, hi = WP, 0
        for c in range(NCORES):
            a = tbound[c] + kc * P
            bnd = min(tbound[c] + (kc + 1) * P, tbound[c + 1])
            if a >= bnd:
                continue
            loc = flat_pid[a:bnd] - wbound[c]
            lo = min(lo, int(loc.min()))
            hi = max(hi, int(loc.max()) + 1)
        wins.append((lo, hi) if lo < hi else None)

    return dict(
        flat_pid=flat_pid, wbound=wbound, tbound=tbound,
        packed_rows=packed_rows, counts=counts, T=T,
        WP=WP, KC=KC, PTW=PTW, wins=wins,
    )


def _mm_plan(plan):
    """Stage-A matmul schedule: per kc, list of (tile, col_lo, col_hi, start).

    Column ranges are relative to the packed axis [0, WP); tile t covers
    [t*PTW, (t+1)*PTW).  The first matmul touching a psum tile streams the
    full tile width with start=True so every element gets initialized.
    """
    WP, PTW, wins, KC = plan["WP"], plan["PTW"], plan["wins"], plan["KC"]
    first = {0: None, 1: None}
    for kc in range(KC):
        if wins[kc] is None:
            continue
        lo, hi = wins[kc]
        for t in (0, 1):
            if lo < (t + 1) * PTW and hi > t * PTW and first[t] is None:
                first[t] = kc
    sched = []
    for kc in range(KC):
        items = []
        if wins[kc] is not None:
            lo, hi = wins[kc]
            for t in (0, 1):
                tl, th = t * PTW, (t + 1) * PTW
                if lo < th and hi > tl:
                    if first[t] == kc:
                        items.append((t, tl, th, True))
                    else:
                        items.append((t, max(lo, tl), min(hi, th), False))
        sched.append(items)
    last = {0: None, 1: None}
    for kc in range(KC):
        for (t, _, _, _) in sched[kc]:
            last[t] = kc
    return sched, last


def _build_program(plan):
    import concourse.mybir as mybir
    from concourse import bacc
    from concourse.tile import TileContext

    bf16 = mybir.dt.bfloat16
    f32 = mybir.dt.float32

    WP, KC, PTW = plan["WP"], plan["KC"], plan["PTW"]
    NWT = WP // P
    sched, last = _mm_plan(plan)

    nc = bacc.Bacc()
    warm_d = nc.dram_tensor("warm", [P, P], bf16, kind="ExternalInput")
    aux_d = nc.dram_tensor("aux", [P, 2, KC], f32, kind="ExternalInput")
    iota_d = nc.dram_tensor("iota", [P, WP], f32, kind="ExternalInput")
    enc_d = nc.dram_tensor("enc", [KC, P, H], bf16, kind="ExternalInput")
    w_d = nc.dram_tensor("wmat", [P, KO, V], bf16, kind="ExternalInput")
    out_d = nc.dram_tensor("out", [WP, V], bf16, kind="ExternalOutput")

    with TileContext(nc) as tc:
        with (
            tc.tile_pool(name="persist", bufs=1) as persist,
            tc.tile_pool(name="wp", bufs=3) as wpool,
            tc.tile_pool(name="outp", bufs=4) as outp,
            tc.tile_pool(name="psA", bufs=4, space="PSUM") as psA,
            tc.tile_pool(name="psB", bufs=4, space="PSUM") as psB,
        ):
            # ---- head: small DMAs first, then enc chunks, then W ----
            warm_sb = persist.tile([P, P], bf16)
            nc.sync.dma_start(out=warm_sb[:], in_=warm_d[:])
            aux_sb = persist.tile([P, 2, KC], f32)
            nc.sync.dma_start(out=aux_sb[:], in_=aux_d[:])
            iota_sb = persist.tile([P, WP], f32)
            nc.sync.dma_start(out=iota_sb[:], in_=iota_d[:])

            enc_sb = persist.tile([P, KC, H], bf16)
            for kc in range(KC):
                nc.sync.dma_start(out=enc_sb[:, kc], in_=enc_d[kc])

            w_tiles = {}

            def load_w(n):
                if n < NPAIR:
                    t = wpool.tile([P, KO, NVP], bf16, tag="w")
                    nc.sync.dma_start(out=t[:], in_=w_d[:, :, n * NVP:(n + 1) * NVP])
                    w_tiles[n] = t

            load_w(0)
            load_w(1)

            # ---- PE warmup on the first tiny DMA ----
            warm_ps = psA.tile([P, PTW], f32, tag="psA")
            for _ in range(NWARM):
                nc.tensor.matmul(
                    warm_ps[:, :64], lhsT=warm_sb[:], rhs=warm_sb[:, :64],
                    start=True, stop=True,
                )

            # ---- onehot tiles (DVE): oneh[tok, col] = (iota==pid)*inv ----
            oneh = {}
            for kc in range(KC):
                if not sched[kc]:
                    continue
                lo = min(cl for (_, cl, _, _) in sched[kc])
                hi = max(ch for (_, _, ch, _) in sched[kc])
                t = persist.tile([P, hi - lo], bf16, name=f"oneh{kc}")
                nc.vector.tensor_scalar(
                    out=t[:],
                    in0=iota_sb[:, lo:hi],
                    scalar1=aux_sb[:, 0, kc:kc + 1],
                    scalar2=aux_sb[:, 1, kc:kc + 1],
                    op0=mybir.AluOpType.is_equal,
                    op1=mybir.AluOpType.mult,
                )
                oneh[kc] = (t, lo)

            # ---- stage A: mergedT[h, packed_w] = enc^T @ onehot ----
            mergedT = persist.tile([P, KO, WP], bf16)
            for ko in range(KO):
                pts = [psA.tile([P, PTW], f32, tag="psA", name=f"pa{ko}_{t}")
                       for t in (0, 1)]
                for kc in range(KC):
                    if not sched[kc]:
                        continue
                    ot, obase = oneh[kc]
                    for (t, cl, ch, st) in sched[kc]:
                        nc.tensor.matmul(
                            pts[t][:, cl - t * PTW:ch - t * PTW],
                            lhsT=enc_sb[:, kc, ko * P:(ko + 1) * P],
                            rhs=ot[:, cl - obase:ch - obase],
                            start=st,
                            stop=(kc == last[t]),
                        )
                for t in (0, 1):
                    nc.vector.tensor_copy(
                        out=mergedT[:, ko, t * PTW:(t + 1) * PTW], in_=pts[t][:],
                    )

            # ---- stage B: out[w, v] = mergedT^T @ W, bf16 out ----
            for n in range(NPAIR):
                load_w(n + 2)
                w_sb = w_tiles.pop(n)
                for wt in range(NWT):
                    st = outp.tile([P, NVP], bf16, tag="out")
                    for hf in range(2):
                        pt = psB.tile([P, 512], f32, tag="psB")
                        for ko in range(KO):
                            nc.tensor.matmul(
                                pt[:],
                                lhsT=mergedT[:, ko, wt * P:(wt + 1) * P],
                                rhs=w_sb[:, ko, hf * 512:(hf + 1) * 512],
                                start=(ko == 0),
                                stop=(ko == KO - 1),
                            )
                        if hf == 0:
                            nc.vector.tensor_copy(
                                out=st[:, :512], in_=pt[:])
                        else:
                            nc.scalar.copy(out=st[:, 512:], in_=pt[:])
                    nc.sync.dma_start(
                        out=out_d[wt * P:(wt + 1) * P, n * NVP:(n + 1) * NVP],
                        in_=st[:],
                    )

    nc.finalize()
    return nc


def _prep_inputs(bert_encodings, W, plan):
    flat_pid, wbound, tbound = plan["flat_pid"], plan["wbound"], plan["tbound"]
    counts, WP, KC = plan["counts"], plan["WP"], plan["KC"]

    enc_bf = np.asarray(bert_encodings, dtype=np.float32).reshape(B * S, H)
    enc_bf = enc_bf.astype(ml_dtypes.bfloat16)
    w_bf = (np.asarray(W, dtype=np.float32).astype(ml_dtypes.bfloat16)
            .reshape(KO, P, V).transpose(1, 0, 2))
    w_bf = np.ascontiguousarray(w_bf)

    inv = (1.0 / counts).astype(np.float32)
    iota = np.ascontiguousarray(
        np.broadcast_to(np.arange(WP, dtype=np.float32), (P, WP)))
    rng = np.random.default_rng(0)
    warm = rng.standard_normal((P, P)).astype(ml_dtypes.bfloat16)

    in_maps = []
    for c in range(NCORES):
        t0, t1 = int(tbound[c]), int(tbound[c + 1])
        ntok = t1 - t0
        enc_c = np.zeros((KC * P, H), dtype=ml_dtypes.bfloat16)
        enc_c[:ntok] = enc_bf[t0:t1]
        enc_c = enc_c.reshape(KC, P, H)

        aux = np.zeros((KC * P, 2), dtype=np.float32)
        aux[:, 0] = -1.0
        pid_loc = (flat_pid[t0:t1] - wbound[c]).astype(np.float32)
        aux[:ntok, 0] = pid_loc
        aux[:ntok, 1] = inv[flat_pid[t0:t1]]
        aux = np.ascontiguousarray(aux.reshape(KC, P, 2).transpose(1, 2, 0))

        in_maps.append({
            "warm": warm, "aux": aux, "iota": iota,
            "enc": enc_c, "wmat": w_bf,
        })
    return in_maps


def kernel(bert_encodings, segment_ids, W, b, num_words, _trace=False):
    from concourse.bass_utils import run_bass_kernel_spmd

    assert int(num_words) == WMAX
    plan = _plan(segment_ids)
    in_maps = _prep_inputs(bert_encodings, W, plan)
    nc = _build_program(plan)

    core_ids = list(range(NCORES))
    res = run_bass_kernel_spmd(nc, in_maps, core_ids, trace=_trace)

    out = np.zeros((B * WMAX, V), dtype=np.float32)
    wbound, packed_rows = plan["wbound"], plan["packed_rows"]
    for c in core_ids:
        nw = int(wbound[c + 1] - wbound[c])
        rows = np.asarray(res.results[c]["out"][:nw]).astype(np.float32)
        out[packed_rows[wbound[c]:wbound[c + 1]]] = rows
    out = out.reshape(B, WMAX, V)

    bias = np.asarray(b, dtype=np.float32)
    if np.any(bias):
        out = out + bias

    if _trace:
        kernel._last_exec_time_ns = res.exec_time_ns
        kernel._last_result = res
    return out
